# revision 4
# baseline (speedup 1.0000x reference)
"""DAHHConv (hypergraph conv) Trainium2 Bass kernel, 8-core SPMD.

Math (reference):
    x' = x @ theta                      # [B,N,C]
    xe = (H^T x') / deg_e               # [B,E,C], deg_e = sum_n H
    xn = (H xe) / deg_n                 # [B,N,C], deg_n = sum_e H
    out = xn + bias

Sharding: 8 cores = 4 batches x 2 halves. Core c handles batch b=c//2,
half h=c%2. Phase 1 (edge aggregation, contraction over n) shards the
E dim: each core owns e in [1024h, 1024h+1024) and needs all N rows
(local, no reduction). Phase 3 (node aggregation, contraction over e)
shards the N dim: each core owns n in [4096h, 4096h+4096) and needs all
E — the xe halves are exchanged with the pair core via a 2-rank
AllGather. H is supplied host-side in BOTH layouts (n-major slice for
phase 1, transposed e-major slice for phase 3) as bf16 — exact, since H
is a 0/1 incidence matrix — halving HBM traffic vs fp32.

deg_e / deg_n come for free as an extra ones-column in the stationary
matmul operands. All divisions are per-partition scalar multiplies
after PE-transposes put e (resp. n) on the partition axis.
"""

import numpy as np
import ml_dtypes

B, N, E, C = 4, 8192, 2048, 64
NCORES = 8
EH = E // 2          # 1024: e-range per core in phase 1
NH = N // 2          # 4096: n-range per core in phase 3
CA = C + 1           # 65: feature dim augmented with ones/deg column
NCHUNK = N // 128    # 64 n-chunks in phase 1
ECHUNK = E // 128    # 16 e-chunks in phase 3
NSPAN = 1024         # phase-3 output span (2 PSUM banks at fp32)
BF16 = ml_dtypes.bfloat16

_cache = {}


def _split_waits_json(raw: bytes) -> bytes:
    """BIR post-pass: this walrus/ISA build allows only ONE sync wait per
    instruction, but the Tile scheduler attaches several. Hoist all but
    the last wait of each instruction onto standalone EventSemaphore
    instructions inserted just before it on the same engine (waits are
    pure preconditions, so running them earlier on the same engine
    stream is equivalent)."""
    import json

    m = json.loads(raw)
    ctr = 0
    for f in m["functions"]:
        for blk in f["blocks"]:
            new = []
            for inst in blk["instructions"]:
                si = inst.get("sync_info")
                waits = (si or {}).get("on_wait") or []
                if len(waits) > 1:
                    for w in waits[:-1]:
                        ctr += 1
                        new.append(
                            {
                                "debug": inst.get("debug", 0),
                                "engine": inst["engine"],
                                "ins": [],
                                "name": f"{inst['name']}-xw{ctr}",
                                "opcode": "EventSemaphore",
                                "outs": [],
                                "sync_info": {"on_update": [], "on_wait": [w]},
                            }
                        )
                    si["on_wait"] = [waits[-1]]
                new.append(inst)
            blk["instructions"] = new
    return json.dumps(m).encode()


def build_bass():
    import concourse.bass as bass
    import concourse.mybir as mybir
    from concourse.tile import TileContext
    from concourse import masks

    dt = mybir.dt
    nc = bass.Bass()

    hn = nc.declare_dram_parameter("hn", [N, EH], dt.bfloat16, isOutput=False)
    ht = nc.declare_dram_parameter("ht", [E, NH], dt.bfloat16, isOutput=False)
    xt = nc.declare_dram_parameter("xt", [C, N], dt.bfloat16, isOutput=False)
    th = nc.declare_dram_parameter("th", [C, C], dt.bfloat16, isOutput=False)
    out = nc.declare_dram_parameter("out", [NH, C], dt.float32, isOutput=True)

    # collective bounce buffers (DRAM; SBUF collectives are banned)
    cc_in = nc.dram_tensor("cc_in", [CA, EH], dt.float32)
    cc_out = nc.dram_tensor("cc_out", [2 * CA, EH], dt.float32)

    with TileContext(nc) as tc:
        with (
            tc.tile_pool(name="const", bufs=1) as const,
            tc.tile_pool(name="persist", bufs=1) as persist,
            tc.tile_pool(name="hn_pool", bufs=8) as hn_pool,
            tc.tile_pool(name="ht_pool", bufs=8) as ht_pool,
            tc.tile_pool(name="small", bufs=2) as small,
        ):
            ident = const.tile([128, 128], dt.float32)
            masks.make_identity(nc, ident[:])
            th_sb = const.tile([C, C], dt.bfloat16)
            nc.sync.dma_start(th_sb[:], th[:])
            xt_sb = persist.tile([C, N], dt.bfloat16)
            nc.sync.dma_start(xt_sb[:], xt[:])

            # x'_aug chunks: chunk j at cols [65j, 65j+65); col 65j+64 = 1
            xp_sb = persist.tile([128, CA * NCHUNK], dt.bfloat16)
            xp_v = xp_sb[:].rearrange("p (c w) -> p c w", w=CA)
            nc.vector.memset(xp_v[:, :, C : C + 1], 1.0)

            # ---- phase 0: x' = x @ theta (theta stationary per chunk) ----
            with tc.tile_pool(name="ps0", bufs=2, space="PSUM") as ps0:
                for blk in range(NCHUNK // 8):
                    ps_xp = ps0.tile([128, 8 * C], dt.float32)
                    for jj in range(8):
                        j = 8 * blk + jj
                        nc.tensor.matmul(
                            ps_xp[:, C * jj : C * (jj + 1)],
                            xt_sb[:, 128 * j : 128 * (j + 1)],
                            th_sb[:],
                        )
                    src = ps_xp[:].rearrange("p (c w) -> p c w", w=C)
                    dst = xp_v[:, 8 * blk : 8 * (blk + 1), 0:C]
                    nc.vector.tensor_copy(dst, src)

            # ---- phase 1: m_e^T[65,1024] = x'_aug^T @ H_n  (accum) ----
            with tc.tile_pool(name="ps1", bufs=1, space="PSUM") as ps1:
                ps_me = ps1.tile([CA, EH], dt.float32)
                for j in range(NCHUNK):
                    hn_t = hn_pool.tile([128, EH], dt.bfloat16)
                    nc.sync.dma_start(hn_t[:], hn[128 * j : 128 * (j + 1), :])
                    for half in range(2):
                        nc.tensor.matmul(
                            ps_me[:, 512 * half : 512 * (half + 1)],
                            xp_sb[:, CA * j : CA * (j + 1)],
                            hn_t[:, 512 * half : 512 * (half + 1)],
                            start=(j == 0),
                            stop=(j == NCHUNK - 1),
                        )
                me_sb = small.tile([CA, EH], dt.float32)
                nc.vector.tensor_copy(me_sb[:], ps_me[:])
                nc.sync.dma_start(cc_in[:], me_sb[:])

            # ---- exchange: 2-rank AllGather within each batch pair ----
            nc.gpsimd.collective_compute(
                "AllGather",
                mybir.AluOpType.bypass,
                replica_groups=[[0, 1], [2, 3], [4, 5], [6, 7]],
                ins=[cc_in[:]],
                outs=[cc_out[:]],
            )

            # ---- phase 2: xe_aug[e,65] = (m_e/deg_e, 1), e on partitions ----
            xe_sb = persist.tile([128, CA * ECHUNK], dt.bfloat16)
            xe_v = xe_sb[:].rearrange("p (c w) -> p c w", w=CA)
            nc.vector.memset(xe_v[:, :, C : C + 1], 1.0)
            with tc.tile_pool(name="ps2", bufs=2, space="PSUM") as ps2:
                for r in range(2):
                    mr = small.tile([CA, EH], dt.float32, tag="mr")
                    nc.sync.dma_start(mr[:], cc_out[CA * r : CA * (r + 1), :])
                    for t in range(EH // 128):
                        k = (EH // 128) * r + t
                        ps_tr = ps2.tile([128, CA], dt.float32)
                        nc.tensor.transpose(
                            ps_tr[:], mr[:, 128 * t : 128 * (t + 1)], ident[0:CA, 0:CA]
                        )
                        rec = small.tile([128, 1], dt.float32, tag="rec")
                        nc.vector.reciprocal(rec[:], ps_tr[:, C : C + 1])
                        nc.vector.tensor_scalar_mul(
                            xe_v[:, k, 0:C], ps_tr[:, 0:C], rec[:]
                        )

            # ---- phase 3: y^T[65,span] = xe_aug^T @ H_e^T; out = y/deg_n ----
            with (
                tc.tile_pool(name="ps3", bufs=2, space="PSUM") as ps3,
                tc.tile_pool(name="ps3t", bufs=2, space="PSUM") as ps3t,
            ):
                for s in range(NH // NSPAN):
                    ps_y = ps3.tile([CA, NSPAN], dt.float32)
                    for k in range(ECHUNK):
                        ht_t = ht_pool.tile([128, NSPAN], dt.bfloat16)
                        nc.sync.dma_start(
                            ht_t[:],
                            ht[128 * k : 128 * (k + 1), NSPAN * s : NSPAN * (s + 1)],
                        )
                        for half in range(2):
                            nc.tensor.matmul(
                                ps_y[:, 512 * half : 512 * (half + 1)],
                                xe_sb[:, CA * k : CA * (k + 1)],
                                ht_t[:, 512 * half : 512 * (half + 1)],
                                start=(k == 0),
                                stop=(k == ECHUNK - 1),
                            )
                    y_sb = small.tile([CA, NSPAN], dt.float32, tag="y_sb")
                    nc.vector.tensor_copy(y_sb[:], ps_y[:])
                    for t in range(NSPAN // 128):
                        ps_t = ps3t.tile([128, CA], dt.float32)
                        nc.tensor.transpose(
                            ps_t[:], y_sb[:, 128 * t : 128 * (t + 1)], ident[0:CA, 0:CA]
                        )
                        rec_n = small.tile([128, 1], dt.float32, tag="rec_n")
                        nc.vector.reciprocal(rec_n[:], ps_t[:, C : C + 1])
                        o_sb = small.tile([128, C], dt.float32, tag="o_sb")
                        nc.vector.tensor_scalar_mul(o_sb[:], ps_t[:, 0:C], rec_n[:])
                        row = NSPAN * s + 128 * t
                        nc.sync.dma_start(out[row : row + 128, :], o_sb[:])

    orig_to_json = nc.to_json_bytes
    nc.to_json_bytes = lambda: _split_waits_json(orig_to_json())
    return nc


def _bf16_exact(a):
    # H is 0/1: truncating the low fp32 mantissa bits is exact.
    return (a.view(np.uint32) >> 16).astype(np.uint16).view(BF16)


def _prepare_in_maps(x, H, theta):
    x = np.ascontiguousarray(x, dtype=np.float32)
    H = np.ascontiguousarray(H, dtype=np.float32)
    th16 = np.ascontiguousarray(theta, dtype=np.float32).astype(BF16)
    in_maps = []
    for c in range(NCORES):
        b, h = divmod(c, 2)
        hn = _bf16_exact(np.ascontiguousarray(H[b, :, EH * h : EH * (h + 1)]))
        ht = _bf16_exact(np.ascontiguousarray(H[b, NH * h : NH * (h + 1), :].T))
        xt = np.ascontiguousarray(x[b].T).astype(BF16)
        in_maps.append({"hn": hn, "ht": ht, "xt": xt, "th": th16})
    return in_maps


def _assemble(results, bias):
    out = np.empty((B, N, C), dtype=np.float32)
    for c in range(NCORES):
        b, h = divmod(c, 2)
        out[b, NH * h : NH * (h + 1), :] = results[c]["out"]
    out += np.asarray(bias, dtype=np.float32)[None, None, :]
    return out


def get_nc():
    if "nc" not in _cache:
        _cache["nc"] = build_bass()
    return _cache["nc"]


def kernel(x, H, theta, bias):
    from concourse.bass_utils import run_bass_kernel_spmd

    nc = get_nc()
    in_maps = _prepare_in_maps(x, H, theta)
    res = run_bass_kernel_spmd(nc, in_maps, list(range(NCORES)))
    return _assemble(res.results, bias)
